# revision 13
# baseline (speedup 1.0000x reference)
"""Distributed TRN2 Bass kernel for nn_ArgmaxISAModule (sparse argmax-attention stack).

Reference (per layer li, fp32):
    KX     = einsum('hqd,dn->hqn', K[li], X)
    scores = einsum('hqn,hqm->hnm', KX, KX)
    mask   = scores >= rowmax(scores) - 0.5
    w      = mask / max(rowsum(mask),1) * (|rowmax| > 0.5)
    attn   = X + sum_h V[li,h] @ (X @ w[h])
    X      = attn + W2[li] @ relu(W1[li] @ attn + b1[li]) + b2[li]

Distribution: token dim n of X sharded across 8 cores (NL=256 columns each).
Per layer: local KX -> AllGather KX (f32, 2MB) -> scores rows for local n in
fp32r on PE -> mask via DVE/Pool/ACT pipeline -> big matmul in transposed
orientation  partial^T[m,d] = sum_{h, n local} w[h][n,m] * (V[h]X*rowscale)^T[n,d]
-> ReduceScatter over m -> PE transpose -> residual + FFN on local columns.

All matmuls run in float32r (TF32-like, 1 cyc/row at free>=256): measured
rel err vs fp32 ~1.5e-4 per matmul, full-model ~3e-3 (gate 2e-2).
"""
import numpy as np

import concourse.bacc as bacc
import concourse.mybir as mybir
import concourse.tile as tile
from concourse import masks
from concourse.bass_utils import run_bass_kernel_spmd

L, H, Q, D, N, DFF = 8, 4, 64, 512, 2048, 2048
CORES = 8
NL = N // CORES          # 256 local columns
P = 128                  # partitions
KD = D // P              # 4 k-tiles over d
KF = DFF // P            # 16 tiles over dff
MT = N // P              # 16 m-tiles (global columns)
F32 = mybir.dt.float32
F32R = mybir.dt.float32r

_cache = {}


def build():
    nc = bacc.Bacc(num_devices=CORES)
    x_in = nc.declare_dram_parameter("x", [D, NL], F32R, isOutput=False)
    kt_in = nc.declare_dram_parameter("kt", [L, D, H * Q], F32R, isOutput=False)
    vt_in = nc.declare_dram_parameter("vt", [L, H, D, D], F32R, isOutput=False)
    w1t_in = nc.declare_dram_parameter("w1t", [L, D, DFF], F32R, isOutput=False)
    b1_in = nc.declare_dram_parameter("b1r", [L, P, KF], F32, isOutput=False)
    w2t_in = nc.declare_dram_parameter("w2t", [L, DFF, D], F32R, isOutput=False)
    b2_in = nc.declare_dram_parameter("b2r", [L, P, KD], F32, isOutput=False)
    out_ext = nc.declare_dram_parameter("out", [D, NL], F32R, isOutput=True)

    HT = [(h, t) for h in range(H) for t in range(NL // P)]  # 8 (head, n-tile)

    from contextlib import ExitStack
    with tile.TileContext(nc) as tc:
        with ExitStack() as stack:
            pool = lambda name, bufs, **kw: stack.enter_context(
                tc.tile_pool(name=name, bufs=bufs, **kw))
            pw = pool("pw", 8)            # w tiles 8x[128,2048]
            pbig = pool("pbig", 2)        # scores_sb
            pkxf = pool("pkxf", 1)        # gathered KX
            pyt = pool("pyt", 8)          # yt / yts
            pstg = pool("pstg", 2)        # big-mm staging
            px = pool("px", 8)            # x tiles
            pat = pool("pat", 4)          # attn tiles
            pff = pool("pff", 1)          # ff1
            pwta = pool("pwta", 6)        # kt/vt streams
            pwtb = pool("pwtb", 3)        # w1f/w2k streams
            pst = pool("pst", 20)         # small stats
            pmisc = pool("pmisc", 1)
            pmisc2 = pool("pmisc2", 2)
            ps_s = pool("ps_s", 2, space="PSUM")   # score chunks [128,512]
            ps_m = pool("ps_m", 2, space="PSUM")   # kx/yt/big/tr/ff [128,<=512]
            ps_x = pool("ps_x", 4, space="PSUM")   # 4 live FFN2 accumulators
            dpool = pool("dram", 2, space="DRAM")

            ident = pmisc.tile([P, P], F32, tag="ident")
            masks.make_identity(nc, ident[:])

            # layer-0 X tiles
            x_tiles = []
            for j in range(KD):
                xt = px.tile([P, NL], F32R, tag="x")
                nc.sync.dma_start(xt[:], x_in[P * j:P * (j + 1), :])
                x_tiles.append(xt)

            for li in range(L):
                # ---- weights for this layer (kt now; others streamed in-phase)
                kt_t = []
                for k in range(KD):
                    t = pwta.tile([P, H * Q], F32R, tag="kt")
                    nc.sync.dma_start(t[:], kt_in[li, P * k:P * (k + 1), :])
                    kt_t.append(t)
                b1_sb = pst.tile([P, KF], F32, tag="b1")
                nc.sync.dma_start(b1_sb[:], b1_in[li])
                b2_sb = pst.tile([P, KD], F32, tag="b2")
                nc.sync.dma_start(b2_sb[:], b2_in[li])

                # ---- KX local: [H*Q, NL] = ktT.T @ X
                kxl = pmisc2.tile([P, 2, NL], F32R, tag="kxl")
                for j in range(2):  # hq partition blocks
                    kx_ps = ps_m.tile([P, NL], F32, tag="mmps")
                    for k in range(KD):
                        nc.tensor.matmul(
                            kx_ps[:], kt_t[k][:, P * j:P * (j + 1)], x_tiles[k][:],
                            start=(k == 0), stop=(k == KD - 1),
                        )
                    nc.vector.tensor_copy(kxl[:, j, :], kx_ps[:])

                # ---- AllGather KX
                ag_in = dpool.tile([H * Q, NL], F32, tag="ag_in")
                ag_out = dpool.tile([N, NL], F32, tag="ag_out")
                nc.sync.dma_start(
                    ag_in[:].rearrange("(j p) n -> p j n", p=P),
                    kxl[:].bitcast(F32),
                )
                nc.gpsimd.collective_compute(
                    "AllGather",
                    mybir.AluOpType.bypass,
                    replica_groups=[list(range(CORES))],
                    ins=[ag_in[:]],
                    outs=[ag_out[:]],
                )
                # kxf[q + 64*(h%2), h//2, m] = KX_full[h, q, m]
                kxf = pkxf.tile([P, 2, N], F32R, tag="kxf")
                ag_v = ag_out[:].rearrange("(c hq) n -> c hq n", c=CORES)
                for h in range(H):
                    po = Q * (h % 2)
                    nc.gpsimd.dma_start(
                        kxf[po:po + Q, h // 2, :].rearrange("q (c n) -> q c n", c=CORES),
                        ag_v[:, Q * h:Q * (h + 1), :].rearrange("c q n -> q c n"),
                    )

                # ---- Yt per (h, t): psum then copy to sbuf (f32)
                yt_sb = {}
                for h in range(H):
                    vt_t = []
                    for k in range(KD):
                        t = pwta.tile([P, D], F32R, tag="vt")
                        nc.sync.dma_start(t[:], vt_in[li, h, P * k:P * (k + 1), :])
                        vt_t.append(t)
                    for t_i in range(NL // P):
                        yp = ps_m.tile([P, D], F32, tag="mmps")
                        for k in range(KD):
                            nc.tensor.matmul(
                                yp[:], x_tiles[k][:, P * t_i:P * (t_i + 1)], vt_t[k][:],
                                start=(k == 0), stop=(k == KD - 1),
                            )
                        ys = pyt.tile([P, D], F32, tag="yt")
                        nc.vector.tensor_copy(ys[:], yp[:])
                        yt_sb[(h, t_i)] = ys

                # ---- scores + mask per (h, t)
                w_tiles = {}
                yts_tiles = {}
                for (h, t_i) in HT:
                    po = Q * (h % 2)
                    lhs = kxl[po:po + Q, h // 2, P * t_i:P * (t_i + 1)]
                    sc_sb = pbig.tile([P, N], F32, tag="scsb")
                    mx = []
                    for c in range(4):
                        sc_ps = ps_s.tile([P, 512], F32, tag="scps")
                        nc.tensor.matmul(
                            sc_ps[:], lhs,
                            kxf[po:po + Q, h // 2, 512 * c:512 * (c + 1)],
                            start=True, stop=True,
                        )
                        nc.scalar.copy(sc_sb[:, 512 * c:512 * (c + 1)], sc_ps[:])
                        m = pst.tile([P, 1], F32, tag="mx")
                        nc.vector.reduce_max(m[:], sc_sb[:, 512 * c:512 * (c + 1)],
                                             axis=mybir.AxisListType.X)
                        mx.append(m)
                    m01 = pst.tile([P, 1], F32, tag="mx01")
                    nc.vector.tensor_tensor(m01[:], mx[0][:], mx[1][:],
                                            op=mybir.AluOpType.max)
                    m23 = pst.tile([P, 1], F32, tag="mx23")
                    nc.vector.tensor_tensor(m23[:], mx[2][:], mx[3][:],
                                            op=mybir.AluOpType.max)
                    rowmax = pst.tile([P, 1], F32, tag="rmax")
                    nc.vector.tensor_tensor(rowmax[:], m01[:], m23[:],
                                            op=mybir.AluOpType.max)
                    thr = pst.tile([P, 1], F32, tag="thr")
                    nc.vector.tensor_scalar_sub(thr[:], rowmax[:], 0.5)
                    # mask + count in one DVE pass
                    wt = pw.tile([P, N], F32R, tag="w")
                    cnt = pst.tile([P, 1], F32, tag="cnt")
                    nc.vector.tensor_scalar(
                        wt[:], sc_sb[:], thr[:], 1.0,
                        mybir.AluOpType.is_ge, mybir.AluOpType.mult,
                        accum_out=cnt[:],
                    )
                    w_tiles[(h, t_i)] = wt
                    # rowscale = (|rowmax| > 0.5) / max(cnt, 1)
                    actp = pst.tile([P, 1], F32, tag="actp")
                    nc.vector.tensor_single_scalar(actp[:], rowmax[:], 0.5,
                                                   op=mybir.AluOpType.is_gt)
                    actn = pst.tile([P, 1], F32, tag="actn")
                    nc.vector.tensor_single_scalar(actn[:], rowmax[:], -0.5,
                                                   op=mybir.AluOpType.is_lt)
                    act = pst.tile([P, 1], F32, tag="act")
                    nc.vector.tensor_tensor(act[:], actp[:], actn[:],
                                            op=mybir.AluOpType.add)
                    nc.vector.tensor_scalar_max(cnt[:], cnt[:], 1.0)
                    rcp = pst.tile([P, 1], F32, tag="rcp")
                    nc.vector.reciprocal(rcp[:], cnt[:])
                    rsc = pst.tile([P, 1], F32, tag="rsc")
                    nc.vector.tensor_tensor(rsc[:], act[:], rcp[:],
                                            op=mybir.AluOpType.mult)
                    yts = pyt.tile([P, D], F32R, tag="yts")
                    nc.vector.tensor_scalar(
                        yts[:], yt_sb[(h, t_i)][:], rsc[:], None,
                        mybir.AluOpType.mult,
                    )
                    yts_tiles[(h, t_i)] = yts

                # ---- big matmul: partial^T[m, d] += w[h][nloc, m].T @ yts[h][nloc, d]
                rs_in = dpool.tile([N, D], F32, tag="rs_in")
                rs_out = dpool.tile([NL, D], F32, tag="rs_out")
                for mt in range(MT):
                    bp = ps_m.tile([P, D], F32, tag="mmps")
                    for i, (h, t_i) in enumerate(HT):
                        nc.tensor.matmul(
                            bp[:], w_tiles[(h, t_i)][:, P * mt:P * (mt + 1)],
                            yts_tiles[(h, t_i)][:],
                            start=(i == 0), stop=(i == len(HT) - 1),
                        )
                    stg = pstg.tile([P, D], F32, tag="bigstg")
                    if mt % 2 == 0:
                        nc.scalar.copy(stg[:], bp[:])
                    else:
                        nc.vector.tensor_copy(stg[:], bp[:])
                    nc.sync.dma_start(rs_in[P * mt:P * (mt + 1), :], stg[:])

                nc.gpsimd.collective_compute(
                    "ReduceScatter",
                    mybir.AluOpType.add,
                    replica_groups=[list(range(CORES))],
                    ins=[rs_in[:]],
                    outs=[rs_out[:]],
                )

                # ---- transpose attn_out^T [NL, D] -> attn [D, NL]; add residual
                attn_t = pmisc.tile([P, NL // P, D], F32, tag="attn_t")
                nc.sync.dma_start(
                    attn_t[:], rs_out[:].rearrange("(t p) d -> p t d", p=P)
                )
                attn_sb = []
                for j in range(KD):
                    a = pat.tile([P, NL], F32R, tag="attn")
                    attn_sb.append(a)
                for t_i in range(NL // P):
                    for j in range(KD):
                        tp = ps_m.tile([P, P], F32, tag="mmps")
                        nc.tensor.transpose(
                            tp[:], attn_t[:, t_i, P * j:P * (j + 1)], ident[:]
                        )
                        nc.vector.scalar_tensor_tensor(
                            attn_sb[j][:, P * t_i:P * (t_i + 1)],
                            tp[:], 1.0, x_tiles[j][:, P * t_i:P * (t_i + 1)].bitcast(F32),
                            op0=mybir.AluOpType.mult, op1=mybir.AluOpType.add,
                        )

                # ---- FFN1: ff1 = relu(W1 @ attn + b1)
                ff1 = pff.tile([P, KF, NL], F32R, tag="ff1")
                for f in range(KF):
                    w1f = pwtb.tile([P, KD, P], F32R, tag="w1f")
                    nc.sync.dma_start(
                        w1f[:],
                        w1t_in[li, :, P * f:P * (f + 1)].rearrange(
                            "(k p) f -> p k f", p=P),
                    )
                    fp = ps_m.tile([P, NL], F32, tag="mmps")
                    for k in range(KD):
                        nc.tensor.matmul(
                            fp[:], w1f[:, k, :], attn_sb[k][:],
                            start=(k == 0), stop=(k == KD - 1),
                        )
                    nc.scalar.activation(
                        ff1[:, f, :], fp[:],
                        mybir.ActivationFunctionType.Relu,
                        bias=b1_sb[:, f:f + 1], scale=1.0,
                    )

                # ---- FFN2 + residuals: X = attn + W2 @ ff1 + b2
                x_ps = []
                for j in range(KD):
                    xps_t = ps_x.tile([P, NL], F32, tag="xps")
                    x_ps.append(xps_t)
                for k in range(KF):
                    w2k = pwtb.tile([P, D], F32R, tag="w2k")
                    nc.sync.dma_start(w2k[:], w2t_in[li, P * k:P * (k + 1), :])
                    for j in range(KD):
                        nc.tensor.matmul(
                            x_ps[j][:], w2k[:, P * j:P * (j + 1)], ff1[:, k, :],
                            start=(k == 0), stop=(k == KF - 1),
                        )
                new_x = []
                for j in range(KD):
                    xn = px.tile([P, NL], F32R, tag="x")
                    nc.vector.scalar_tensor_tensor(
                        xn[:], x_ps[j][:], b2_sb[:, j:j + 1],
                        attn_sb[j][:].bitcast(F32),
                        op0=mybir.AluOpType.add, op1=mybir.AluOpType.add,
                    )
                    new_x.append(xn)
                x_tiles = new_x

            for j in range(KD):
                nc.sync.dma_start(out_ext[P * j:P * (j + 1), :], x_tiles[j][:])

    nc.finalize()
    return nc


def kernel(**inputs) -> np.ndarray:
    X = np.ascontiguousarray(inputs["X"], dtype=np.float32)
    K = np.asarray(inputs["K"], dtype=np.float32)
    V = np.asarray(inputs["V"], dtype=np.float32)
    W1 = np.asarray(inputs["W1"], dtype=np.float32)
    b1 = np.asarray(inputs["b1"], dtype=np.float32)
    W2 = np.asarray(inputs["W2"], dtype=np.float32)
    b2 = np.asarray(inputs["b2"], dtype=np.float32)

    kt = np.ascontiguousarray(K.reshape(L, H * Q, D).transpose(0, 2, 1))
    vt = np.ascontiguousarray(V.transpose(0, 1, 3, 2))
    w1t = np.ascontiguousarray(W1.transpose(0, 2, 1))
    w2t = np.ascontiguousarray(W2.transpose(0, 2, 1))
    b1r = np.ascontiguousarray(b1.reshape(L, KF, P).transpose(0, 2, 1))
    b2r = np.ascontiguousarray(b2.reshape(L, KD, P).transpose(0, 2, 1))

    if "nc" not in _cache:
        _cache["nc"] = build()
    nc = _cache["nc"]

    in_maps = []
    for c in range(CORES):
        in_maps.append({
            "x": np.ascontiguousarray(X[:, c * NL:(c + 1) * NL]),
            "kt": kt, "vt": vt, "w1t": w1t, "b1r": b1r,
            "w2t": w2t, "b2r": b2r,
        })
    res = run_bass_kernel_spmd(nc, in_maps, core_ids=list(range(CORES)))
    out = np.concatenate([res.results[c]["out"] for c in range(CORES)], axis=1)
    return out.astype(np.float32)


if __name__ == "__main__":
    rng = np.random.default_rng(0)
    print("smoke build only")
    build()
    print("build ok")
